# revision 4
# baseline (speedup 1.0000x reference)
"""Bass TRN2 kernel for nn_PennyLaneHead (10-qubit VQC head).

Math: out = (|U @ psi0(x)|^2) @ G + b, where
  - angles = tanh(x @ W_proj.T) * pi/2; psi0 = real product state from
    cos/sin of half-angles (qubit 0 = MSB in C-order flatten)
  - U = fixed 1024x1024 unitary of the entangling circuit (depends only
    on `weights`) -> constant-folded on host in complex128
  - G[amp, c] = Zsigns.T @ W_out.T folds PauliZ expvals + output layer

Device work per core (1024 samples): angles matmuls, tanh/sin, product
state build, PE transposes, two 1024x1024 real matmuls (re/im), square-
add, and a 1024->10 output matmul.
"""

import os
import numpy as np
from contextlib import ExitStack

import concourse.bass as bass
import concourse.tile as tile
from concourse import bacc, mybir
from concourse.bass_utils import run_bass_kernel_spmd

N = 10
DEPTH = 6
B_FULL = 8192
F = 1024
C = 10
NCORES = 8
BS = B_FULL // NCORES          # 1024 samples per core
AMP = 1 << N                   # 1024 amplitudes
FP32 = mybir.dt.float32
AF = mybir.ActivationFunctionType
PI_2 = float(np.pi / 2)
PI_4 = float(np.pi / 4)

LAST_EXEC_NS = None


def _build_unitary(weights):
    """Return UT (1024,1024) complex128 with UT[k, j] = U[j, k]."""
    w = np.asarray(weights, np.float64)
    psi = np.eye(AMP, dtype=np.complex128).reshape((AMP,) + (2,) * N)

    def apply_1q(psi, U, q):
        psi = np.tensordot(U, psi, axes=([1], [q + 1]))
        return np.moveaxis(psi, 0, q + 1)

    def apply_cnot(psi, c, t):
        psi = np.moveaxis(psi, (c + 1, t + 1), (1, 2))
        psi = np.concatenate([psi[:, :1], psi[:, 1:, ::-1]], axis=1)
        return np.moveaxis(psi, (1, 2), (c + 1, t + 1))

    def rot(phi, theta, omega):
        c, s = np.cos(theta / 2), np.sin(theta / 2)
        ep = np.exp(-0.5j * (phi + omega))
        em = np.exp(-0.5j * (phi - omega))
        return np.array([[ep * c, -np.conj(em) * s], [em * s, np.conj(ep) * c]])

    for l in range(DEPTH):
        for i in range(N):
            psi = apply_1q(psi, rot(w[l, i, 0], w[l, i, 1], w[l, i, 2]), i)
        r = (l % (N - 1)) + 1
        for i in range(N):
            psi = apply_cnot(psi, i, (i + r) % N)
    return psi.reshape(AMP, AMP)


def _build_program():
    nc = bacc.Bacc("TRN2", target_bir_lowering=False, debug=False,
                   num_devices=NCORES)

    xt_d = nc.dram_tensor("xt", (F, BS), FP32, kind="ExternalInput").ap()
    utr_d = nc.dram_tensor("utr", (AMP, AMP), FP32, kind="ExternalInput").ap()
    uti_d = nc.dram_tensor("uti", (AMP, AMP), FP32, kind="ExternalInput").ap()
    wpt_d = nc.dram_tensor("wpt", (F, N), FP32, kind="ExternalInput").ap()
    g_d = nc.dram_tensor("g", (AMP, C), FP32, kind="ExternalInput").ap()
    b_d = nc.dram_tensor("b", (C, 1), FP32, kind="ExternalInput").ap()
    id_d = nc.dram_tensor("ident", (128, 128), FP32, kind="ExternalInput").ap()
    out_d = nc.dram_tensor("out", (C, BS), FP32, kind="ExternalOutput").ap()

    NS = 512                    # samples per big group
    NG = BS // NS               # 2 groups
    NT = BS // 128              # 8 sample sub-tiles

    with tile.TileContext(nc) as tc, ExitStack() as ctx:
        const = ctx.enter_context(tc.tile_pool(name="const", bufs=1))

        xt_sb = const.tile([128, 8 * BS], FP32, tag="xt")
        utr_sb = const.tile([128, 8 * AMP], FP32, tag="utr")
        uti_sb = const.tile([128, 8 * AMP], FP32, tag="uti")
        wpt_sb = const.tile([128, 8 * N], FP32, tag="wpt")
        g_sb = const.tile([128, 8 * C], FP32, tag="g")
        b_sb = const.tile([C, 1], FP32, tag="b")
        id_sb = const.tile([128, 128], FP32, tag="ident")
        tanhT = const.tile([N, BS], FP32, tag="tanhT")
        pi2_sb = const.tile([128, 1], FP32, tag="pi2")
        nc.vector.memset(pi2_sb[:], PI_2)

        nc.gpsimd.dma_start(b_sb[:], b_d[:])
        nc.gpsimd.dma_start(id_sb[:], id_d[:])
        for k8 in range(8):
            nc.gpsimd.dma_start(xt_sb[:, bass.ts(k8, BS)],
                                xt_d[bass.ts(k8, 128), :])
            nc.gpsimd.dma_start(wpt_sb[:, bass.ts(k8, N)],
                                wpt_d[bass.ts(k8, 128), :])
            nc.gpsimd.dma_start(g_sb[:, bass.ts(k8, C)],
                                g_d[bass.ts(k8, 128), :])
        for k8 in range(8):
            nc.gpsimd.dma_start(utr_sb[:, bass.ts(k8, AMP)],
                                utr_d[bass.ts(k8, 128), :])
            nc.gpsimd.dma_start(uti_sb[:, bass.ts(k8, AMP)],
                                uti_d[bass.ts(k8, 128), :])

        psiT_pool = ctx.enter_context(tc.tile_pool(name="psiT", bufs=2))
        psiTs = []

        # ---- Phase A: angles + product states + transposes -------------
        with tc.tile_pool(name="psA", bufs=2, space=bass.MemorySpace.PSUM) \
                as psA, \
             tc.tile_pool(name="sbA", bufs=2) as sbA:
            for h in range(2):
                pre_ps = psA.tile([N, BS // 2], FP32, tag="pre")
                for k8 in range(8):
                    nc.tensor.matmul(
                        pre_ps[:],
                        wpt_sb[:, bass.ts(k8, N)],
                        xt_sb[:, k8 * BS + h * (BS // 2):
                              k8 * BS + (h + 1) * (BS // 2)],
                        start=(k8 == 0), stop=(k8 == 7))
                nc.scalar.activation(tanhT[:, bass.ts(h, BS // 2)], pre_ps[:],
                                     AF.Tanh)

            for t in range(NT):
                if t % 4 == 0:
                    psiT = psiT_pool.tile([128, 8 * NS], FP32, tag="psiT")
                    psiTs.append(psiT)
                toff = (t % 4) * 128

                ang_ps = psA.tile([128, N], FP32, tag="ang")
                nc.tensor.transpose(ang_ps[:], tanhT[:, bass.ts(t, 128)],
                                    id_sb[0:N, 0:N])
                c_sb = sbA.tile([128, N], FP32, tag="c")
                s_sb = sbA.tile([128, N], FP32, tag="s")
                nc.scalar.activation(c_sb[:], ang_ps[:], AF.Sin,
                                     bias=pi2_sb[:], scale=PI_4)
                nc.scalar.activation(s_sb[:], ang_ps[:], AF.Sin,
                                     bias=0.0, scale=PI_4)

                psi = sbA.tile([128, AMP], FP32, tag="psi")
                nc.vector.tensor_copy(psi[:, 0:1], c_sb[:, 9:10])
                nc.vector.tensor_copy(psi[:, 1:2], s_sb[:, 9:10])
                for q in range(8, -1, -1):
                    L = 1 << (9 - q)
                    nc.vector.tensor_scalar_mul(psi[:, L:2 * L], psi[:, 0:L],
                                                s_sb[:, q:q + 1])
                    nc.vector.tensor_scalar_mul(psi[:, 0:L], psi[:, 0:L],
                                                c_sb[:, q:q + 1])

                for k8 in range(8):
                    tr_ps = psA.tile([128, 128], FP32, tag="tr")
                    nc.tensor.transpose(tr_ps[:], psi[:, bass.ts(k8, 128)],
                                        id_sb[:])
                    dst = psiT[:, k8 * NS + toff: k8 * NS + toff + 128]
                    if k8 % 2 == 0:
                        nc.vector.tensor_copy(dst, tr_ps[:])
                    else:
                        nc.scalar.activation(dst, tr_ps[:], AF.Copy)

        # ---- Phase B: U matmuls + |.|^2 + output ----------------------
        with tc.tile_pool(name="psB", bufs=2, space=bass.MemorySpace.PSUM) \
                as psB, \
             tc.tile_pool(name="sbB", bufs=2) as sbB:
            for g in range(NG):
                psiT = psiTs[g]
                oacc = sbB.tile([C, NS], FP32, tag="oacc")
                for j8 in range(8):
                    re_ps = psB.tile([128, NS], FP32, tag="re")
                    im_ps = psB.tile([128, NS], FP32, tag="im")
                    for k8 in range(8):
                        nc.tensor.matmul(
                            re_ps[:],
                            utr_sb[:, k8 * AMP + j8 * 128:
                                   k8 * AMP + (j8 + 1) * 128],
                            psiT[:, bass.ts(k8, NS)],
                            start=(k8 == 0), stop=(k8 == 7))
                    for k8 in range(8):
                        nc.tensor.matmul(
                            im_ps[:],
                            uti_sb[:, k8 * AMP + j8 * 128:
                                   k8 * AMP + (j8 + 1) * 128],
                            psiT[:, bass.ts(k8, NS)],
                            start=(k8 == 0), stop=(k8 == 7))
                    sq_re = sbB.tile([128, NS], FP32, tag="sqre")
                    sq_im = sbB.tile([128, NS], FP32, tag="sqim")
                    nc.scalar.activation(sq_re[:], re_ps[:], AF.Square)
                    nc.scalar.activation(sq_im[:], im_ps[:], AF.Square)
                    pt = sbB.tile([128, NS], FP32, tag="pt")
                    nc.vector.tensor_add(pt[:], sq_re[:], sq_im[:])

                    o_ps = psB.tile([C, NS], FP32, tag="o")
                    nc.tensor.matmul(o_ps[:], g_sb[:, bass.ts(j8, C)], pt[:],
                                     start=True, stop=True)
                    if j8 == 0:
                        nc.vector.tensor_scalar_add(oacc[:], o_ps[:],
                                                    b_sb[:, 0:1])
                    else:
                        nc.vector.tensor_add(oacc[:], oacc[:], o_ps[:])
                nc.gpsimd.dma_start(out_d[:, bass.ts(g, NS)], oacc[:])

    nc.compile()
    return nc


_PROG = None


def kernel(x, W_proj, weights, W_out, b_out):
    global _PROG, LAST_EXEC_NS
    x = np.asarray(x, np.float32)
    W_proj = np.asarray(W_proj, np.float32)
    W_out = np.asarray(W_out, np.float32)
    b_out = np.asarray(b_out, np.float32)

    UT = _build_unitary(weights)
    utr = np.ascontiguousarray(UT.real.astype(np.float32))
    uti = np.ascontiguousarray(UT.imag.astype(np.float32))

    bits = (np.arange(AMP)[None, :] >> (N - 1 - np.arange(N)[:, None])) & 1
    zs = (1.0 - 2.0 * bits).astype(np.float32)            # (10, 1024)
    g = np.ascontiguousarray(zs.T @ W_out.T)              # (1024, 10)
    wpt = np.ascontiguousarray(W_proj.T)                  # (1024, 10)
    b = np.ascontiguousarray(b_out.reshape(C, 1))
    ident = np.eye(128, dtype=np.float32)
    xt = np.ascontiguousarray(x.T)                        # (1024, 8192)

    if _PROG is None:
        _PROG = _build_program()

    in_maps = []
    for c in range(NCORES):
        in_maps.append({
            "xt": np.ascontiguousarray(xt[:, c * BS:(c + 1) * BS]),
            "utr": utr, "uti": uti, "wpt": wpt, "g": g, "b": b,
            "ident": ident,
        })

    want_trace = bool(int(os.environ.get("KERNEL_TRACE", "0")))
    try:
        rr = run_bass_kernel_spmd(
            _PROG, in_maps, core_ids=list(range(NCORES)), trace=want_trace)
    except Exception:
        if not want_trace:
            raise
        rr = run_bass_kernel_spmd(
            _PROG, in_maps, core_ids=list(range(NCORES)), trace=False)
    LAST_EXEC_NS = rr.exec_time_ns

    outT = np.concatenate([np.asarray(r["out"]) for r in rr.results], axis=1)
    return np.ascontiguousarray(outT.T).astype(np.float32)


# revision 25
# speedup vs baseline: 1.6765x; 1.6765x over previous
"""Bass TRN2 kernel for nn_PennyLaneHead (10-qubit VQC head).

Math: out = (|U @ psi0(x)|^2) @ G + b, where
  - angles = tanh(x @ W_proj.T) * pi/2; psi0 = real product state from
    cos/sin of half-angles (qubit 0 = MSB in C-order flatten)
  - U = fixed 1024x1024 unitary of the entangling circuit (depends only
    on `weights`) -> constant-folded on host in complex128
  - G[amp, c] = Zsigns.T @ W_out.T folds PauliZ expvals + output layer

Device work per core (1024 samples): angles matmuls, tanh/sin, product
state build, PE transposes, two 1024x1024 real matmuls (re/im), square-
add, and a 1024->10 output matmul. The U matmuls + psi chain run in
bf16 (1 PE cycle/row vs 4 for fp32, half the U DMA bytes); angles and
the |.|^2 / output path stay fp32.
"""

import os
import numpy as np
from contextlib import ExitStack

import concourse.bass as bass
import concourse.tile as tile
from concourse import bacc, mybir
from concourse.bass_utils import run_bass_kernel_spmd

N = 10
DEPTH = 6
B_FULL = 8192
F = 1024
C = 10
NCORES = 8
BS = B_FULL // NCORES          # 1024 samples per core
AMP = 1 << N                   # 1024 amplitudes
FP32 = mybir.dt.float32
FP16 = mybir.dt.float16
AF = mybir.ActivationFunctionType
PI_2 = float(np.pi / 2)
PI_4 = float(np.pi / 4)

LAST_EXEC_NS = None
# fp16 for the U matmuls: 1 PE cycle/row like bf16, but 10 mantissa
# bits; all operands are <= 1 in magnitude so fp16 range is a non-issue
USE_FP16 = os.environ.get("KERNEL_FP16", "1") == "1"
DT_U = FP16 if USE_FP16 else FP32


def _build_unitary(weights):
    """Return UT (1024,1024) complex128 with UT[k, j] = U[j, k]."""
    w = np.asarray(weights, np.float64)
    psi = np.eye(AMP, dtype=np.complex128).reshape((AMP,) + (2,) * N)

    def apply_1q(psi, U, q):
        psi = np.tensordot(U, psi, axes=([1], [q + 1]))
        return np.moveaxis(psi, 0, q + 1)

    def apply_cnot(psi, c, t):
        psi = np.moveaxis(psi, (c + 1, t + 1), (1, 2))
        psi = np.concatenate([psi[:, :1], psi[:, 1:, ::-1]], axis=1)
        return np.moveaxis(psi, (1, 2), (c + 1, t + 1))

    def rot(phi, theta, omega):
        c, s = np.cos(theta / 2), np.sin(theta / 2)
        ep = np.exp(-0.5j * (phi + omega))
        em = np.exp(-0.5j * (phi - omega))
        return np.array([[ep * c, -np.conj(em) * s], [em * s, np.conj(ep) * c]])

    for l in range(DEPTH):
        for i in range(N):
            psi = apply_1q(psi, rot(w[l, i, 0], w[l, i, 1], w[l, i, 2]), i)
        r = (l % (N - 1)) + 1
        for i in range(N):
            psi = apply_cnot(psi, i, (i + r) % N)
    return psi.reshape(AMP, AMP)


def _build_program():
    nc = bacc.Bacc("TRN2", target_bir_lowering=False, debug=False,
                   num_devices=NCORES)

    xt_d = nc.dram_tensor("xt", (F, BS), FP32, kind="ExternalInput").ap()
    utr_d = nc.dram_tensor("utr", (AMP, AMP), DT_U, kind="ExternalInput").ap()
    uti_d = nc.dram_tensor("uti", (AMP, AMP), DT_U, kind="ExternalInput").ap()
    wpt_d = nc.dram_tensor("wpt", (F, N), FP32, kind="ExternalInput").ap()
    g_d = nc.dram_tensor("g", (AMP, C), FP32, kind="ExternalInput").ap()
    b_d = nc.dram_tensor("b", (C, 1), FP32, kind="ExternalInput").ap()
    id_d = nc.dram_tensor("ident", (128, 128), FP32, kind="ExternalInput").ap()
    out_d = nc.dram_tensor("out", (C, BS), FP32, kind="ExternalOutput").ap()

    NS = 512                    # samples per big group
    NG = BS // NS               # 2 groups
    NT = BS // 128              # 8 sample sub-tiles

    with tile.TileContext(nc) as tc, ExitStack() as ctx:
        const = ctx.enter_context(tc.tile_pool(name="const", bufs=1))

        xt_sb = const.tile([128, 8 * BS], FP32, tag="xt")
        utr_sb = const.tile([128, 8 * AMP], DT_U, tag="utr")
        uti_sb = const.tile([128, 8 * AMP], DT_U, tag="uti")
        wpt_sb = const.tile([128, 8 * N], FP32, tag="wpt")
        g_sb = const.tile([128, 8 * C], FP32, tag="g")
        b_sb = const.tile([C, 1], FP32, tag="b")
        id_sb = const.tile([128, 128], FP32, tag="ident")
        tanhT = const.tile([N, BS], FP32, tag="tanhT")
        pi2_sb = const.tile([128, 1], FP32, tag="pi2")
        nc.vector.memset(pi2_sb[:], PI_2)

        nc.gpsimd.dma_start(b_sb[:], b_d[:])
        nc.gpsimd.dma_start(id_sb[:], id_d[:])
        for k8 in range(8):
            nc.gpsimd.dma_start(xt_sb[:, bass.ts(k8, BS)],
                                xt_d[bass.ts(k8, 128), :])
            nc.gpsimd.dma_start(wpt_sb[:, bass.ts(k8, N)],
                                wpt_d[bass.ts(k8, 128), :])
            nc.gpsimd.dma_start(g_sb[:, bass.ts(k8, C)],
                                g_d[bass.ts(k8, 128), :])
        for k8 in range(8):
            nc.gpsimd.dma_start(utr_sb[:, bass.ts(k8, AMP)],
                                utr_d[bass.ts(k8, 128), :])
            nc.gpsimd.dma_start(uti_sb[:, bass.ts(k8, AMP)],
                                uti_d[bass.ts(k8, 128), :])

        psiT_pool = ctx.enter_context(tc.tile_pool(name="psiT", bufs=2))
        psiTs = []

        # ---- Phase A: angles + product states + transposes -------------
        with tc.tile_pool(name="psA", bufs=2, space=bass.MemorySpace.PSUM) \
                as psA, \
             tc.tile_pool(name="sbA", bufs=2) as sbA:
            for h in range(2):
                pre_ps = psA.tile([N, BS // 2], FP32, tag="pre")
                for k8 in range(8):
                    nc.tensor.matmul(
                        pre_ps[:],
                        wpt_sb[:, bass.ts(k8, N)],
                        xt_sb[:, k8 * BS + h * (BS // 2):
                              k8 * BS + (h + 1) * (BS // 2)],
                        start=(k8 == 0), stop=(k8 == 7))
                nc.scalar.activation(tanhT[:, bass.ts(h, BS // 2)], pre_ps[:],
                                     AF.Tanh)

            for t in range(NT):
                if t % 4 == 0:
                    psiT = psiT_pool.tile([128, 8 * NS], DT_U, tag="psiT")
                    psiTs.append(psiT)
                toff = (t % 4) * 128

                ang_ps = psA.tile([128, N], FP32, tag="ang")
                nc.tensor.transpose(ang_ps[:], tanhT[:, bass.ts(t, 128)],
                                    id_sb[0:N, 0:N])
                c_sb = sbA.tile([128, N], FP32, tag="c")
                s_sb = sbA.tile([128, N], FP32, tag="s")
                nc.scalar.activation(c_sb[:], ang_ps[:], AF.Sin,
                                     bias=pi2_sb[:], scale=PI_4)
                nc.scalar.activation(s_sb[:], ang_ps[:], AF.Sin,
                                     bias=0.0, scale=PI_4)

                psi = sbA.tile([128, AMP], FP32, tag="psi")
                nc.vector.tensor_copy(psi[:, 0:1], c_sb[:, 9:10])
                nc.vector.tensor_copy(psi[:, 1:2], s_sb[:, 9:10])
                for q in range(8, -1, -1):
                    L = 1 << (9 - q)
                    nc.vector.tensor_scalar_mul(psi[:, L:2 * L], psi[:, 0:L],
                                                s_sb[:, q:q + 1])
                    nc.vector.tensor_scalar_mul(psi[:, 0:L], psi[:, 0:L],
                                                c_sb[:, q:q + 1])

                for k8 in range(8):
                    tr_ps = psA.tile([128, 128], FP32, tag="tr")
                    nc.tensor.transpose(tr_ps[:], psi[:, bass.ts(k8, 128)],
                                        id_sb[:])
                    dst = psiT[:, k8 * NS + toff: k8 * NS + toff + 128]
                    if k8 % 2 == 0:
                        nc.vector.tensor_copy(dst, tr_ps[:])
                    else:
                        nc.scalar.activation(dst, tr_ps[:], AF.Copy)

        # ---- Phase B: U matmuls + |.|^2 + output ----------------------
        with tc.tile_pool(name="psB", bufs=2, space=bass.MemorySpace.PSUM) \
                as psB, \
             tc.tile_pool(name="sbB", bufs=2) as sbB:
            for g in range(NG):
                psiT = psiTs[g]
                oacc = sbB.tile([C, NS], FP32, tag="oacc")
                for j8 in range(8):
                    re_ps = psB.tile([128, NS], FP32, tag="re")
                    im_ps = psB.tile([128, NS], FP32, tag="im")
                    for k8 in range(8):
                        nc.tensor.matmul(
                            re_ps[:],
                            utr_sb[:, k8 * AMP + j8 * 128:
                                   k8 * AMP + (j8 + 1) * 128],
                            psiT[:, bass.ts(k8, NS)],
                            start=(k8 == 0), stop=(k8 == 7))
                    for k8 in range(8):
                        nc.tensor.matmul(
                            im_ps[:],
                            uti_sb[:, k8 * AMP + j8 * 128:
                                   k8 * AMP + (j8 + 1) * 128],
                            psiT[:, bass.ts(k8, NS)],
                            start=(k8 == 0), stop=(k8 == 7))
                    sq_re = sbB.tile([128, NS], FP32, tag="sqre")
                    sq_im = sbB.tile([128, NS], FP32, tag="sqim")
                    nc.scalar.activation(sq_re[:], re_ps[:], AF.Square)
                    nc.scalar.activation(sq_im[:], im_ps[:], AF.Square)
                    pt = sbB.tile([128, NS], FP32, tag="pt")
                    nc.vector.tensor_add(pt[:], sq_re[:], sq_im[:])

                    o_ps = psB.tile([C, NS], FP32, tag="o")
                    nc.tensor.matmul(o_ps[:], g_sb[:, bass.ts(j8, C)],
                                     pt[:], start=True, stop=True)
                    if j8 == 0:
                        nc.vector.tensor_scalar_add(oacc[:], o_ps[:],
                                                    b_sb[:, 0:1])
                    else:
                        nc.vector.tensor_add(oacc[:], oacc[:], o_ps[:])
                nc.gpsimd.dma_start(out_d[:, bass.ts(g, NS)], oacc[:])

    nc.compile()
    return nc


_PROG = None


def kernel(x, W_proj, weights, W_out, b_out):
    global _PROG, LAST_EXEC_NS
    x = np.asarray(x, np.float32)
    W_proj = np.asarray(W_proj, np.float32)
    W_out = np.asarray(W_out, np.float32)
    b_out = np.asarray(b_out, np.float32)

    UT = _build_unitary(weights)
    u_np = np.float16 if USE_FP16 else np.float32
    utr = np.ascontiguousarray(UT.real.astype(u_np))
    uti = np.ascontiguousarray(UT.imag.astype(u_np))

    bits = (np.arange(AMP)[None, :] >> (N - 1 - np.arange(N)[:, None])) & 1
    zs = (1.0 - 2.0 * bits).astype(np.float32)            # (10, 1024)
    g = np.ascontiguousarray(zs.T @ W_out.T)              # (1024, 10)
    wpt = np.ascontiguousarray(W_proj.T)                  # (1024, 10)
    b = np.ascontiguousarray(b_out.reshape(C, 1))
    ident = np.eye(128, dtype=np.float32)
    xt = np.ascontiguousarray(x.T)                        # (1024, 8192)

    if _PROG is None:
        _PROG = _build_program()

    in_maps = []
    for c in range(NCORES):
        in_maps.append({
            "xt": np.ascontiguousarray(xt[:, c * BS:(c + 1) * BS]),
            "utr": utr, "uti": uti, "wpt": wpt, "g": g, "b": b,
            "ident": ident,
        })

    want_trace = bool(int(os.environ.get("KERNEL_TRACE", "0")))
    try:
        rr = run_bass_kernel_spmd(
            _PROG, in_maps, core_ids=list(range(NCORES)), trace=want_trace)
    except Exception:
        if not want_trace:
            raise
        rr = run_bass_kernel_spmd(
            _PROG, in_maps, core_ids=list(range(NCORES)), trace=False)
    LAST_EXEC_NS = rr.exec_time_ns

    outT = np.concatenate([np.asarray(r["out"]) for r in rr.results], axis=1)
    return np.ascontiguousarray(outT.T).astype(np.float32)


# revision 34
# speedup vs baseline: 2.2155x; 1.3216x over previous
"""Bass TRN2 kernel for nn_PennyLaneHead (10-qubit VQC head).

Math: out = (|U @ psi0(x)|^2) @ G + b, where
  - angles = tanh(x @ W_proj.T) * pi/2; psi0 = real product state from
    cos/sin of half-angles (qubit 0 = MSB in C-order flatten)
  - U = fixed 1024x1024 unitary of the entangling circuit (depends only
    on `weights`) -> constant-folded on host in complex128
  - G[amp, c] = Zsigns.T @ W_out.T folds PauliZ expvals + output layer

psi0 is built in the log domain to keep everything on PE/ACT:
  log|psi0[k]| = sum_q bit_q(k)*(ln sin|h_q| - ln cos h_q) + sum_q ln cos h_q
computed as one K=20 matmul (split fp16 hi+lo for f32-level accuracy),
sign(psi0[k]) = cos(pi * sum_q bit_q(k)*[h_q<0]) via a K=10 matmul + Sin.
This avoids the serial per-sample product chain + PE transposes entirely.
"""

import os
import numpy as np
from contextlib import ExitStack

import concourse.bass as bass
import concourse.tile as tile
from concourse import bacc, mybir
from concourse.bass_utils import run_bass_kernel_spmd

N = 10
DEPTH = 6
B_FULL = 8192
F = 1024
C = 10
NCORES = 8
BS = B_FULL // NCORES          # 1024 samples per core
AMP = 1 << N                   # 1024 amplitudes
NS = 512                       # batch group width (1 PSUM bank for f32)
NG = BS // NS                  # 2 groups
FP32 = mybir.dt.float32
FP16 = mybir.dt.float16
AF = mybir.ActivationFunctionType
PI = float(np.pi)
PI_2 = float(np.pi / 2)
PI_4 = float(np.pi / 4)
EPS = 1e-6                     # ln(sin|h| + EPS): bounds log at ~-13.8

LAST_EXEC_NS = None
USE_FP16 = True


def _build_unitary(weights):
    """Return UT (1024,1024) complex128 with UT[k, j] = U[j, k]."""
    w = np.asarray(weights, np.float64)
    psi = np.eye(AMP, dtype=np.complex128).reshape((AMP,) + (2,) * N)

    def apply_1q(psi, U, q):
        psi = np.tensordot(U, psi, axes=([1], [q + 1]))
        return np.moveaxis(psi, 0, q + 1)

    def apply_cnot(psi, c, t):
        psi = np.moveaxis(psi, (c + 1, t + 1), (1, 2))
        psi = np.concatenate([psi[:, :1], psi[:, 1:, ::-1]], axis=1)
        return np.moveaxis(psi, (1, 2), (c + 1, t + 1))

    def rot(phi, theta, omega):
        c, s = np.cos(theta / 2), np.sin(theta / 2)
        ep = np.exp(-0.5j * (phi + omega))
        em = np.exp(-0.5j * (phi - omega))
        return np.array([[ep * c, -np.conj(em) * s], [em * s, np.conj(ep) * c]])

    for l in range(DEPTH):
        for i in range(N):
            psi = apply_1q(psi, rot(w[l, i, 0], w[l, i, 1], w[l, i, 2]), i)
        r = (l % (N - 1)) + 1
        for i in range(N):
            psi = apply_cnot(psi, i, (i + r) % N)
    return psi.reshape(AMP, AMP)


def _build_program():
    nc = bacc.Bacc("TRN2", target_bir_lowering=False, debug=False,
                   num_devices=NCORES)

    xt_d = nc.dram_tensor("xt", (F, BS), FP32, kind="ExternalInput").ap()
    utr_d = nc.dram_tensor("utr", (AMP, AMP), FP16, kind="ExternalInput").ap()
    uti_d = nc.dram_tensor("uti", (AMP, AMP), FP16, kind="ExternalInput").ap()
    wpt_d = nc.dram_tensor("wpt", (F, N), FP32, kind="ExternalInput").ap()
    g_d = nc.dram_tensor("g", (AMP, C), FP16, kind="ExternalInput").ap()
    b_d = nc.dram_tensor("b", (C, 1), FP32, kind="ExternalInput").ap()
    el_d = nc.dram_tensor("el", (128, AMP), FP16, kind="ExternalInput").ap()
    out_d = nc.dram_tensor("out", (C, BS), FP32, kind="ExternalOutput").ap()

    with tile.TileContext(nc) as tc, ExitStack() as ctx:
        const = ctx.enter_context(tc.tile_pool(name="const", bufs=1))

        xt_sb = const.tile([128, 8 * BS], FP32, tag="xt")
        utr_sb = const.tile([128, 8 * AMP], FP16, tag="utr")
        uti_sb = const.tile([128, 8 * AMP], FP16, tag="uti")
        wpt_sb = const.tile([128, 8 * N], FP32, tag="wpt")
        g_sb = const.tile([128, 8 * C], FP16, tag="g")
        b_sb = const.tile([C, 1], FP32, tag="b")
        el_sb = const.tile([128, AMP], FP16, tag="el")
        psiT = const.tile([128, 8 * BS], FP16, tag="psiT")
        pi2_sb = const.tile([128, 1], FP32, tag="pi2")
        nc.vector.memset(pi2_sb[:], PI_2)
        eps_sb = const.tile([128, 1], FP32, tag="eps")
        nc.vector.memset(eps_sb[:], EPS)

        nc.gpsimd.dma_start(b_sb[:], b_d[:])
        nc.gpsimd.dma_start(el_sb[:], el_d[:])
        for k8 in range(8):
            nc.gpsimd.dma_start(xt_sb[:, bass.ts(k8, BS)],
                                xt_d[bass.ts(k8, 128), :])
            nc.gpsimd.dma_start(wpt_sb[:, bass.ts(k8, N)],
                                wpt_d[bass.ts(k8, 128), :])
            nc.gpsimd.dma_start(g_sb[:, bass.ts(k8, C)],
                                g_d[bass.ts(k8, 128), :])
        for k8 in range(8):
            nc.gpsimd.dma_start(utr_sb[:, bass.ts(k8, AMP)],
                                utr_d[bass.ts(k8, 128), :])
            nc.gpsimd.dma_start(uti_sb[:, bass.ts(k8, AMP)],
                                uti_d[bass.ts(k8, 128), :])

        PS = bass.MemorySpace.PSUM
        psPre = ctx.enter_context(tc.tile_pool(name="psPre", bufs=2, space=PS))
        psLP = ctx.enter_context(tc.tile_pool(name="psLP", bufs=1, space=PS))
        psB = ctx.enter_context(tc.tile_pool(name="psB", bufs=1, space=PS))
        psO = ctx.enter_context(tc.tile_pool(name="psO", bufs=2, space=PS))
        sbA = ctx.enter_context(tc.tile_pool(name="sbA", bufs=2))
        sbB = ctx.enter_context(tc.tile_pool(name="sbB", bufs=2))

        def gcols(k8, g):
            return slice(k8 * BS + g * NS, k8 * BS + (g + 1) * NS)

        def phase_a(g):
            pre_ps = psPre.tile([N, NS], FP32, tag="pre")
            for k8 in range(8):
                nc.tensor.matmul(pre_ps[:], wpt_sb[:, bass.ts(k8, N)],
                                 xt_sb[:, gcols(k8, g)],
                                 start=(k8 == 0), stop=(k8 == 7))
            th = sbA.tile([N, NS], FP32, tag="th")
            nc.scalar.activation(th[:], pre_ps[:], AF.Tanh)
            c = sbA.tile([N, NS], FP32, tag="c")
            nc.scalar.activation(c[:], th[:], AF.Sin, bias=pi2_sb[0:N, :],
                                 scale=PI_4)
            lc = sbA.tile([N, NS], FP32, tag="lc")
            nc.scalar.activation(lc[:], c[:], AF.Ln)
            ab = sbA.tile([N, NS], FP32, tag="ab")
            nc.scalar.activation(ab[:], th[:], AF.Abs, scale=PI_4)
            sa = sbA.tile([N, NS], FP32, tag="sa")
            nc.scalar.activation(sa[:], ab[:], AF.Sin)
            ls = sbA.tile([N, NS], FP32, tag="ls")
            nc.scalar.activation(ls[:], sa[:], AF.Ln, bias=eps_sb[0:N, :])
            nn = sbA.tile([N, NS], FP32, tag="nn")
            nc.scalar.activation(nn[:], th[:], AF.Sign)
            nh = sbA.tile([N, NS], FP16, tag="nh")
            nc.scalar.activation(nh[:], nn[:], AF.Copy, bias=0.5, scale=-0.5)
            # dl128 quadrant layout (matches el row blocks): [0:10] hi of
            # ls-lc, [32:42] hi of lc, [64:74] lo of ls-lc, [96:106] lo of lc
            lsmc = sbA.tile([N, NS], FP32, tag="lsmc")
            nc.vector.tensor_sub(lsmc[:], ls[:], lc[:])
            dl = sbA.tile([128, NS], FP16, tag="dl")
            nc.vector.memset(dl[:], 0.0)
            nc.vector.tensor_copy(dl[0:N, :], lsmc[:])
            nc.vector.tensor_copy(dl[32:32 + N, :], lc[:])
            hi_a = sbA.tile([N, NS], FP32, tag="hi_a")
            nc.vector.tensor_copy(hi_a[:], dl[0:N, :])
            hi_b = sbA.tile([N, NS], FP32, tag="hi_b")
            nc.vector.tensor_copy(hi_b[:], dl[32:32 + N, :])
            nc.vector.tensor_sub(dl[64:64 + N, :], lsmc[:], hi_a[:])
            nc.vector.tensor_sub(dl[96:96 + N, :], lc[:], hi_b[:])
            return dl, nh

        def phase_lp(g, dl, nh):
            for k8 in range(8):
                L_ps = psLP.tile([128, NS], FP32, tag="L")
                nc.tensor.matmul(L_ps[:], el_sb[:, bass.ts(k8, 128)], dl[:],
                                 start=True, stop=True)
                P_ps = psLP.tile([128, NS], FP32, tag="P")
                nc.tensor.matmul(P_ps[:], el_sb[0:N, bass.ts(k8, 128)], nh[:],
                                 start=True, stop=True)
                eL = sbB.tile([128, NS], FP16, tag="eL")
                nc.scalar.activation(eL[:], L_ps[:], AF.Exp)
                # Exact parity: fp16 round-to-even at 2048 keeps only even
                # integers, so d = (P+2048) - fp16(P+2048) = +-(P mod 2)
                # and sign = 1 - 2*d^2 = (-1)^P.
                p16 = sbB.tile([128, NS], FP16, tag="p16")
                nc.scalar.activation(p16[:], P_ps[:], AF.Copy, bias=2048.0)
                d = sbB.tile([128, NS], FP32, tag="d")
                nc.vector.scalar_tensor_tensor(
                    d[:], P_ps[:], 2048.0, p16[:],
                    mybir.AluOpType.add, mybir.AluOpType.subtract)
                d2 = sbB.tile([128, NS], FP32, tag="d2")
                nc.vector.tensor_mul(d2[:], d[:], d[:])
                sgn = sbB.tile([128, NS], FP16, tag="sgn")
                nc.vector.tensor_scalar(sgn[:], d2[:], -2.0, 1.0,
                                        mybir.AluOpType.mult,
                                        mybir.AluOpType.add)
                nc.vector.tensor_mul(psiT[:, gcols(k8, g)], sgn[:], eL[:])

        def phase_b(g):
            o_ps = psO.tile([C, NS], FP32, tag="o")
            pts = []

            def issue_o(j8):
                nc.tensor.matmul(o_ps[:], g_sb[:, bass.ts(j8, C)],
                                 pts[j8][:], start=(j8 == 0), stop=(j8 == 7))

            for j8 in range(8):
                re_ps = psB.tile([128, NS], FP32, tag="re")
                for k8 in range(8):
                    nc.tensor.matmul(
                        re_ps[:],
                        utr_sb[:, k8 * AMP + j8 * 128:
                               k8 * AMP + (j8 + 1) * 128],
                        psiT[:, gcols(k8, g)],
                        start=(k8 == 0), stop=(k8 == 7))
                im_ps = psB.tile([128, NS], FP32, tag="im")
                for k8 in range(8):
                    nc.tensor.matmul(
                        im_ps[:],
                        uti_sb[:, k8 * AMP + j8 * 128:
                               k8 * AMP + (j8 + 1) * 128],
                        psiT[:, gcols(k8, g)],
                        start=(k8 == 0), stop=(k8 == 7))
                sq_re = sbB.tile([128, NS], FP16, tag="sqre")
                nc.scalar.activation(sq_re[:], re_ps[:], AF.Square)
                sq_im = sbB.tile([128, NS], FP16, tag="sqim")
                nc.scalar.activation(sq_im[:], im_ps[:], AF.Square)
                pt = sbB.tile([128, NS], FP16, tag="pt")
                nc.vector.tensor_add(pt[:], sq_re[:], sq_im[:])
                pts.append(pt)
                if j8 >= 1:
                    issue_o(j8 - 1)
            issue_o(7)
            osb = sbB.tile([C, NS], FP32, tag="osb")
            nc.vector.tensor_scalar_add(osb[:], o_ps[:], b_sb[:, 0:1])
            nc.gpsimd.dma_start(out_d[:, bass.ts(g, NS)], osb[:])

        dln = [phase_a(g) for g in range(NG)]
        for g in range(NG):
            phase_lp(g, *dln[g])
        for g in range(NG):
            phase_b(g)

    nc.compile()
    return nc


def _host_arrays(x, W_proj, weights, W_out, b_out):
    UT = _build_unitary(weights)
    utr = np.ascontiguousarray(UT.real.astype(np.float16))
    uti = np.ascontiguousarray(UT.imag.astype(np.float16))

    bits = (np.arange(AMP)[None, :] >> (N - 1 - np.arange(N)[:, None])) & 1
    zs = (1.0 - 2.0 * bits).astype(np.float32)            # (10, 1024)
    g = np.ascontiguousarray(
        (zs.T @ W_out.T.astype(np.float32)).astype(np.float16))
    el = np.zeros((128, AMP), np.float16)
    el[0:N] = bits
    el[32:32 + N] = 1.0
    el[64:64 + N] = bits
    el[96:96 + N] = 1.0
    el = np.ascontiguousarray(el)
    wpt = np.ascontiguousarray(W_proj.T)                  # (1024, 10)
    b = np.ascontiguousarray(b_out.reshape(C, 1))
    xt = np.ascontiguousarray(x.T)                        # (1024, 8192)
    return dict(xt=xt, utr=utr, uti=uti, wpt=wpt, g=g, b=b, el=el)


_PROG = None


def kernel(x, W_proj, weights, W_out, b_out):
    global _PROG, LAST_EXEC_NS
    x = np.asarray(x, np.float32)
    W_proj = np.asarray(W_proj, np.float32)
    W_out = np.asarray(W_out, np.float32)
    b_out = np.asarray(b_out, np.float32)

    h = _host_arrays(x, W_proj, weights, W_out, b_out)
    xt = h.pop("xt")

    if _PROG is None:
        _PROG = _build_program()

    in_maps = []
    for ci in range(NCORES):
        m = dict(h)
        m["xt"] = np.ascontiguousarray(xt[:, ci * BS:(ci + 1) * BS])
        in_maps.append(m)

    want_trace = bool(int(os.environ.get("KERNEL_TRACE", "0")))
    try:
        rr = run_bass_kernel_spmd(
            _PROG, in_maps, core_ids=list(range(NCORES)), trace=want_trace)
    except Exception:
        if not want_trace:
            raise
        rr = run_bass_kernel_spmd(
            _PROG, in_maps, core_ids=list(range(NCORES)), trace=False)
    LAST_EXEC_NS = rr.exec_time_ns

    outT = np.concatenate([np.asarray(r["out"]) for r in rr.results], axis=1)
    return np.ascontiguousarray(outT.T).astype(np.float32)


# revision 39
# speedup vs baseline: 2.2901x; 1.0337x over previous
"""Bass TRN2 kernel for nn_PennyLaneHead (10-qubit VQC head).

Math: out = (|U @ psi0(x)|^2) @ G + b, where
  - angles = tanh(x @ W_proj.T) * pi/2; psi0 = real product state from
    cos/sin of half-angles (qubit 0 = MSB in C-order flatten)
  - U = fixed 1024x1024 unitary of the entangling circuit (depends only
    on `weights`) -> constant-folded on host in complex128
  - G[amp, c] = Zsigns.T @ W_out.T folds PauliZ expvals + output layer

psi0 is built in the log domain to keep everything on PE/ACT:
  log|psi0[k]| = sum_q bit_q(k)*(ln sin|h_q| - ln cos h_q) + sum_q ln cos h_q
computed as one K=20 matmul (split fp16 hi+lo for f32-level accuracy),
sign(psi0[k]) = cos(pi * sum_q bit_q(k)*[h_q<0]) via a K=10 matmul + Sin.
This avoids the serial per-sample product chain + PE transposes entirely.
"""

import os
import numpy as np
from contextlib import ExitStack

import concourse.bass as bass
import concourse.tile as tile
from concourse import bacc, mybir
from concourse.bass_utils import run_bass_kernel_spmd

N = 10
DEPTH = 6
B_FULL = 8192
F = 1024
C = 10
NCORES = 8
BS = B_FULL // NCORES          # 1024 samples per core
AMP = 1 << N                   # 1024 amplitudes
NS = 512                       # batch group width (1 PSUM bank for f32)
NG = BS // NS                  # 2 groups
FP32 = mybir.dt.float32
FP16 = mybir.dt.float16
AF = mybir.ActivationFunctionType
PI = float(np.pi)
PI_2 = float(np.pi / 2)
PI_4 = float(np.pi / 4)
EPS = 1e-6                     # ln(sin|h| + EPS): bounds log at ~-13.8

LAST_EXEC_NS = None
USE_FP16 = True


def _build_unitary(weights):
    """Return UT (1024,1024) complex128 with UT[k, j] = U[j, k]."""
    w = np.asarray(weights, np.float64)
    psi = np.eye(AMP, dtype=np.complex128).reshape((AMP,) + (2,) * N)

    def apply_1q(psi, U, q):
        psi = np.tensordot(U, psi, axes=([1], [q + 1]))
        return np.moveaxis(psi, 0, q + 1)

    def apply_cnot(psi, c, t):
        psi = np.moveaxis(psi, (c + 1, t + 1), (1, 2))
        psi = np.concatenate([psi[:, :1], psi[:, 1:, ::-1]], axis=1)
        return np.moveaxis(psi, (1, 2), (c + 1, t + 1))

    def rot(phi, theta, omega):
        c, s = np.cos(theta / 2), np.sin(theta / 2)
        ep = np.exp(-0.5j * (phi + omega))
        em = np.exp(-0.5j * (phi - omega))
        return np.array([[ep * c, -np.conj(em) * s], [em * s, np.conj(ep) * c]])

    for l in range(DEPTH):
        for i in range(N):
            psi = apply_1q(psi, rot(w[l, i, 0], w[l, i, 1], w[l, i, 2]), i)
        r = (l % (N - 1)) + 1
        for i in range(N):
            psi = apply_cnot(psi, i, (i + r) % N)
    return psi.reshape(AMP, AMP)


def _build_program():
    nc = bacc.Bacc("TRN2", target_bir_lowering=False, debug=False,
                   num_devices=NCORES)

    xt_d = nc.dram_tensor("xt", (F, BS), FP32, kind="ExternalInput").ap()
    utr_d = nc.dram_tensor("utr", (AMP, AMP), FP16, kind="ExternalInput").ap()
    uti_d = nc.dram_tensor("uti", (AMP, AMP), FP16, kind="ExternalInput").ap()
    wpt_d = nc.dram_tensor("wpt", (F, N), FP32, kind="ExternalInput").ap()
    g_d = nc.dram_tensor("g", (AMP, C), FP16, kind="ExternalInput").ap()
    b_d = nc.dram_tensor("b", (C, 1), FP32, kind="ExternalInput").ap()
    el_d = nc.dram_tensor("el", (128, AMP), FP16, kind="ExternalInput").ap()
    out_d = nc.dram_tensor("out", (C, BS), FP32, kind="ExternalOutput").ap()

    with tile.TileContext(nc) as tc, ExitStack() as ctx:
        const = ctx.enter_context(tc.tile_pool(name="const", bufs=1))

        xt_sb = const.tile([128, 8 * BS], FP32, tag="xt")
        utr_sb = const.tile([128, 8 * AMP], FP16, tag="utr")
        uti_sb = const.tile([128, 8 * AMP], FP16, tag="uti")
        wpt_sb = const.tile([128, 8 * N], FP32, tag="wpt")
        g_sb = const.tile([128, 8 * C], FP16, tag="g")
        b_sb = const.tile([C, 1], FP32, tag="b")
        el_sb = const.tile([128, AMP], FP16, tag="el")
        psiT = const.tile([128, 8 * BS], FP16, tag="psiT")
        pi2_sb = const.tile([128, 1], FP32, tag="pi2")
        nc.vector.memset(pi2_sb[:], PI_2)
        eps_sb = const.tile([128, 1], FP32, tag="eps")
        nc.vector.memset(eps_sb[:], EPS)

        nc.gpsimd.dma_start(b_sb[:], b_d[:])
        nc.gpsimd.dma_start(el_sb[:], el_d[:])
        for k8 in range(8):
            nc.gpsimd.dma_start(xt_sb[:, bass.ts(k8, BS)],
                                xt_d[bass.ts(k8, 128), :])
            nc.gpsimd.dma_start(wpt_sb[:, bass.ts(k8, N)],
                                wpt_d[bass.ts(k8, 128), :])
            nc.gpsimd.dma_start(g_sb[:, bass.ts(k8, C)],
                                g_d[bass.ts(k8, 128), :])
        for k8 in range(8):
            nc.gpsimd.dma_start(utr_sb[:, bass.ts(k8, AMP)],
                                utr_d[bass.ts(k8, 128), :])
            nc.gpsimd.dma_start(uti_sb[:, bass.ts(k8, AMP)],
                                uti_d[bass.ts(k8, 128), :])

        PS = bass.MemorySpace.PSUM
        psPre = ctx.enter_context(tc.tile_pool(name="psPre", bufs=1, space=PS))
        psL = ctx.enter_context(tc.tile_pool(name="psL", bufs=2, space=PS))
        psP = ctx.enter_context(tc.tile_pool(name="psP", bufs=2, space=PS))
        psB = ctx.enter_context(tc.tile_pool(name="psB", bufs=1, space=PS))
        psO = ctx.enter_context(tc.tile_pool(name="psO", bufs=1, space=PS))
        sbA = ctx.enter_context(tc.tile_pool(name="sbA", bufs=2))
        sbB = ctx.enter_context(tc.tile_pool(name="sbB", bufs=2))

        def gcols(k8, g):
            return slice(k8 * BS + g * NS, k8 * BS + (g + 1) * NS)

        def phase_a(g):
            pre_ps = psPre.tile([N, NS], FP32, tag="pre")
            for k8 in range(8):
                nc.tensor.matmul(pre_ps[:], wpt_sb[:, bass.ts(k8, N)],
                                 xt_sb[:, gcols(k8, g)],
                                 start=(k8 == 0), stop=(k8 == 7))
            th = sbA.tile([N, NS], FP32, tag="th")
            nc.scalar.activation(th[:], pre_ps[:], AF.Tanh)
            c = sbA.tile([N, NS], FP32, tag="c")
            nc.scalar.activation(c[:], th[:], AF.Sin, bias=pi2_sb[0:N, :],
                                 scale=PI_4)
            lc = sbA.tile([N, NS], FP32, tag="lc")
            nc.scalar.activation(lc[:], c[:], AF.Ln)
            ab = sbA.tile([N, NS], FP32, tag="ab")
            nc.scalar.activation(ab[:], th[:], AF.Abs, scale=PI_4)
            sa = sbA.tile([N, NS], FP32, tag="sa")
            nc.scalar.activation(sa[:], ab[:], AF.Sin)
            ls = sbA.tile([N, NS], FP32, tag="ls")
            nc.scalar.activation(ls[:], sa[:], AF.Ln, bias=eps_sb[0:N, :])
            nn = sbA.tile([N, NS], FP32, tag="nn")
            nc.scalar.activation(nn[:], th[:], AF.Sign)
            nh = sbA.tile([N, NS], FP16, tag="nh")
            nc.scalar.activation(nh[:], nn[:], AF.Copy, bias=0.5, scale=-0.5)
            # dl128 quadrant layout (matches el row blocks): [0:10] hi of
            # ls-lc, [32:42] hi of lc, [64:74] lo of ls-lc, [96:106] lo of lc
            lsmc = sbA.tile([N, NS], FP32, tag="lsmc")
            nc.vector.tensor_sub(lsmc[:], ls[:], lc[:])
            dl = sbA.tile([128, NS], FP16, tag="dl")
            nc.vector.memset(dl[:], 0.0)
            nc.vector.tensor_copy(dl[0:N, :], lsmc[:])
            nc.vector.tensor_copy(dl[32:32 + N, :], lc[:])
            hi_a = sbA.tile([N, NS], FP32, tag="hi_a")
            nc.vector.tensor_copy(hi_a[:], dl[0:N, :])
            hi_b = sbA.tile([N, NS], FP32, tag="hi_b")
            nc.vector.tensor_copy(hi_b[:], dl[32:32 + N, :])
            nc.vector.tensor_sub(dl[64:64 + N, :], lsmc[:], hi_a[:])
            nc.vector.tensor_sub(dl[96:96 + N, :], lc[:], hi_b[:])
            return dl, nh

        def phase_lp_k8(g, dl, nh, k8):
                L_ps = psL.tile([128, NS], FP32, tag="L")
                nc.tensor.matmul(L_ps[:], el_sb[:, bass.ts(k8, 128)], dl[:],
                                 start=True, stop=True)
                P_ps = psP.tile([128, NS], FP32, tag="P")
                nc.tensor.matmul(P_ps[:], el_sb[0:N, bass.ts(k8, 128)], nh[:],
                                 start=True, stop=True)
                eL = sbB.tile([128, NS], FP16, tag="eL")
                nc.scalar.activation(eL[:], L_ps[:], AF.Exp)
                # Exact parity: fp16 round-to-even at 2048 keeps only even
                # integers, so d = (P+2048) - fp16(P+2048) = +-(P mod 2)
                # and sign = 1 - 2*d^2 = (-1)^P.
                p16 = sbB.tile([128, NS], FP16, tag="p16")
                nc.scalar.activation(p16[:], P_ps[:], AF.Copy, bias=2048.0)
                d = sbB.tile([128, NS], FP32, tag="d")
                nc.vector.scalar_tensor_tensor(
                    d[:], P_ps[:], 2048.0, p16[:],
                    mybir.AluOpType.add, mybir.AluOpType.subtract)
                d2 = sbB.tile([128, NS], FP32, tag="d2")
                nc.vector.tensor_mul(d2[:], d[:], d[:])
                sgn = sbB.tile([128, NS], FP16, tag="sgn")
                nc.vector.tensor_scalar(sgn[:], d2[:], -2.0, 1.0,
                                        mybir.AluOpType.mult,
                                        mybir.AluOpType.add)
                nc.vector.tensor_mul(psiT[:, gcols(k8, g)], sgn[:], eL[:])

        def phase_b(g, post_j8=None):
            o_ps = psO.tile([C, NS], FP32, tag="o")
            pts = []

            def issue_o(j8):
                nc.tensor.matmul(o_ps[:], g_sb[:, bass.ts(j8, C)],
                                 pts[j8][:], start=(j8 == 0), stop=(j8 == 7))

            for j8 in range(8):
                re_ps = psB.tile([128, NS], FP32, tag="re")
                for k8 in range(8):
                    nc.tensor.matmul(
                        re_ps[:],
                        utr_sb[:, k8 * AMP + j8 * 128:
                               k8 * AMP + (j8 + 1) * 128],
                        psiT[:, gcols(k8, g)],
                        start=(k8 == 0), stop=(k8 == 7))
                im_ps = psB.tile([128, NS], FP32, tag="im")
                for k8 in range(8):
                    nc.tensor.matmul(
                        im_ps[:],
                        uti_sb[:, k8 * AMP + j8 * 128:
                               k8 * AMP + (j8 + 1) * 128],
                        psiT[:, gcols(k8, g)],
                        start=(k8 == 0), stop=(k8 == 7))
                sq_re = sbB.tile([128, NS], FP16, tag="sqre")
                nc.scalar.activation(sq_re[:], re_ps[:], AF.Square)
                sq_im = sbB.tile([128, NS], FP16, tag="sqim")
                nc.scalar.activation(sq_im[:], im_ps[:], AF.Square)
                pt = sbB.tile([128, NS], FP16, tag="pt")
                nc.vector.tensor_add(pt[:], sq_re[:], sq_im[:])
                pts.append(pt)
                if j8 >= 1:
                    issue_o(j8 - 1)
                if post_j8 is not None:
                    post_j8(j8)
            issue_o(7)
            osb = sbB.tile([C, NS], FP32, tag="osb")
            nc.vector.tensor_scalar_add(osb[:], o_ps[:], b_sb[:, 0:1])
            nc.gpsimd.dma_start(out_d[:, bass.ts(g, NS)], osb[:])

        dln = [phase_a(g) for g in range(NG)]
        for k8 in range(8):
            phase_lp_k8(0, *dln[0], k8)
        phase_b(0, post_j8=lambda j8: phase_lp_k8(1, *dln[1], j8))
        phase_b(1)

    nc.compile()
    return nc


def _host_arrays(x, W_proj, weights, W_out, b_out):
    UT = _build_unitary(weights)
    utr = np.ascontiguousarray(UT.real.astype(np.float16))
    uti = np.ascontiguousarray(UT.imag.astype(np.float16))

    bits = (np.arange(AMP)[None, :] >> (N - 1 - np.arange(N)[:, None])) & 1
    zs = (1.0 - 2.0 * bits).astype(np.float32)            # (10, 1024)
    g = np.ascontiguousarray(
        (zs.T @ W_out.T.astype(np.float32)).astype(np.float16))
    el = np.zeros((128, AMP), np.float16)
    el[0:N] = bits
    el[32:32 + N] = 1.0
    el[64:64 + N] = bits
    el[96:96 + N] = 1.0
    el = np.ascontiguousarray(el)
    wpt = np.ascontiguousarray(W_proj.T)                  # (1024, 10)
    b = np.ascontiguousarray(b_out.reshape(C, 1))
    xt = np.ascontiguousarray(x.T)                        # (1024, 8192)
    return dict(xt=xt, utr=utr, uti=uti, wpt=wpt, g=g, b=b, el=el)


_PROG = None


def kernel(x, W_proj, weights, W_out, b_out):
    global _PROG, LAST_EXEC_NS
    x = np.asarray(x, np.float32)
    W_proj = np.asarray(W_proj, np.float32)
    W_out = np.asarray(W_out, np.float32)
    b_out = np.asarray(b_out, np.float32)

    h = _host_arrays(x, W_proj, weights, W_out, b_out)
    xt = h.pop("xt")

    if _PROG is None:
        _PROG = _build_program()

    in_maps = []
    for ci in range(NCORES):
        m = dict(h)
        m["xt"] = np.ascontiguousarray(xt[:, ci * BS:(ci + 1) * BS])
        in_maps.append(m)

    want_trace = bool(int(os.environ.get("KERNEL_TRACE", "0")))
    try:
        rr = run_bass_kernel_spmd(
            _PROG, in_maps, core_ids=list(range(NCORES)), trace=want_trace)
    except Exception:
        if not want_trace:
            raise
        rr = run_bass_kernel_spmd(
            _PROG, in_maps, core_ids=list(range(NCORES)), trace=False)
    LAST_EXEC_NS = rr.exec_time_ns

    outT = np.concatenate([np.asarray(r["out"]) for r in rr.results], axis=1)
    return np.ascontiguousarray(outT.T).astype(np.float32)


# revision 42
# speedup vs baseline: 2.3049x; 1.0064x over previous
"""Bass TRN2 kernel for nn_PennyLaneHead (10-qubit VQC head).

Math: out = (|U @ psi0(x)|^2) @ G + b, where
  - angles = tanh(x @ W_proj.T) * pi/2; psi0 = real product state from
    cos/sin of half-angles (qubit 0 = MSB in C-order flatten)
  - U = fixed 1024x1024 unitary of the entangling circuit (depends only
    on `weights`) -> constant-folded on host in complex128
  - G[amp, c] = Zsigns.T @ W_out.T folds PauliZ expvals + output layer

psi0 is built in the log domain to keep everything on PE/ACT:
  log|psi0[k]| = sum_q bit_q(k)*(ln sin|h_q| - ln cos h_q) + sum_q ln cos h_q
computed as one K=20 matmul (split fp16 hi+lo for f32-level accuracy),
sign(psi0[k]) = cos(pi * sum_q bit_q(k)*[h_q<0]) via a K=10 matmul + Sin.
This avoids the serial per-sample product chain + PE transposes entirely.
"""

import os
import numpy as np
from contextlib import ExitStack

import concourse.bass as bass
import concourse.tile as tile
from concourse import bacc, mybir
from concourse.bass_utils import run_bass_kernel_spmd

N = 10
DEPTH = 6
B_FULL = 8192
F = 1024
C = 10
NCORES = 8
BS = B_FULL // NCORES          # 1024 samples per core
AMP = 1 << N                   # 1024 amplitudes
NS = 512                       # batch group width (1 PSUM bank for f32)
NG = BS // NS                  # 2 groups
FP32 = mybir.dt.float32
FP16 = mybir.dt.float16
AF = mybir.ActivationFunctionType
PI = float(np.pi)
PI_2 = float(np.pi / 2)
PI_4 = float(np.pi / 4)
EPS = 1e-6                     # ln(sin|h| + EPS): bounds log at ~-13.8

LAST_EXEC_NS = None
USE_FP16 = True


def _build_unitary(weights):
    """Return UT (1024,1024) complex128 with UT[k, j] = U[j, k]."""
    w = np.asarray(weights, np.float64)
    psi = np.eye(AMP, dtype=np.complex128).reshape((AMP,) + (2,) * N)

    def apply_1q(psi, U, q):
        psi = np.tensordot(U, psi, axes=([1], [q + 1]))
        return np.moveaxis(psi, 0, q + 1)

    def apply_cnot(psi, c, t):
        psi = np.moveaxis(psi, (c + 1, t + 1), (1, 2))
        psi = np.concatenate([psi[:, :1], psi[:, 1:, ::-1]], axis=1)
        return np.moveaxis(psi, (1, 2), (c + 1, t + 1))

    def rot(phi, theta, omega):
        c, s = np.cos(theta / 2), np.sin(theta / 2)
        ep = np.exp(-0.5j * (phi + omega))
        em = np.exp(-0.5j * (phi - omega))
        return np.array([[ep * c, -np.conj(em) * s], [em * s, np.conj(ep) * c]])

    for l in range(DEPTH):
        for i in range(N):
            psi = apply_1q(psi, rot(w[l, i, 0], w[l, i, 1], w[l, i, 2]), i)
        r = (l % (N - 1)) + 1
        for i in range(N):
            psi = apply_cnot(psi, i, (i + r) % N)
    return psi.reshape(AMP, AMP)


def _build_program():
    nc = bacc.Bacc("TRN2", target_bir_lowering=False, debug=False,
                   num_devices=NCORES)

    xt_d = nc.dram_tensor("xt", (F, BS), FP32, kind="ExternalInput").ap()
    utr_d = nc.dram_tensor("utr", (AMP, AMP), FP16, kind="ExternalInput").ap()
    uti_d = nc.dram_tensor("uti", (AMP, AMP), FP16, kind="ExternalInput").ap()
    wpt_d = nc.dram_tensor("wpt", (F, N), FP32, kind="ExternalInput").ap()
    g_d = nc.dram_tensor("g", (AMP, C), FP16, kind="ExternalInput").ap()
    b_d = nc.dram_tensor("b", (C, 1), FP32, kind="ExternalInput").ap()
    el_d = nc.dram_tensor("el", (128, AMP), FP16, kind="ExternalInput").ap()
    out_d = nc.dram_tensor("out", (C, BS), FP32, kind="ExternalOutput").ap()

    with tile.TileContext(nc) as tc, ExitStack() as ctx:
        const = ctx.enter_context(tc.tile_pool(name="const", bufs=1))

        xt_sb = const.tile([128, 8 * BS], FP32, tag="xt")
        utr_sb = const.tile([128, 8 * AMP], FP16, tag="utr")
        uti_sb = const.tile([128, 8 * AMP], FP16, tag="uti")
        wpt_sb = const.tile([128, 8 * N], FP32, tag="wpt")
        g_sb = const.tile([128, 8 * C], FP16, tag="g")
        b_sb = const.tile([C, 1], FP32, tag="b")
        el_sb = const.tile([128, AMP], FP16, tag="el")
        psiT = const.tile([128, 8 * BS], FP16, tag="psiT")
        pi2_sb = const.tile([128, 1], FP32, tag="pi2")
        nc.vector.memset(pi2_sb[:], PI_2)
        eps_sb = const.tile([128, 1], FP32, tag="eps")
        nc.vector.memset(eps_sb[:], EPS)

        nc.gpsimd.dma_start(b_sb[:], b_d[:])
        nc.gpsimd.dma_start(el_sb[:], el_d[:])
        for k8 in range(8):
            nc.gpsimd.dma_start(xt_sb[:, bass.ts(k8, BS)],
                                xt_d[bass.ts(k8, 128), :])
            nc.gpsimd.dma_start(wpt_sb[:, bass.ts(k8, N)],
                                wpt_d[bass.ts(k8, 128), :])
            nc.gpsimd.dma_start(g_sb[:, bass.ts(k8, C)],
                                g_d[bass.ts(k8, 128), :])
        for k8 in range(8):
            nc.gpsimd.dma_start(utr_sb[:, bass.ts(k8, AMP)],
                                utr_d[bass.ts(k8, 128), :])
            nc.gpsimd.dma_start(uti_sb[:, bass.ts(k8, AMP)],
                                uti_d[bass.ts(k8, 128), :])

        PS = bass.MemorySpace.PSUM
        psPre = ctx.enter_context(tc.tile_pool(name="psPre", bufs=1, space=PS))
        psL = ctx.enter_context(tc.tile_pool(name="psL", bufs=2, space=PS))
        psP = ctx.enter_context(tc.tile_pool(name="psP", bufs=2, space=PS))
        psB = ctx.enter_context(tc.tile_pool(name="psB", bufs=1, space=PS))
        psO = ctx.enter_context(tc.tile_pool(name="psO", bufs=1, space=PS))
        sbA = ctx.enter_context(tc.tile_pool(name="sbA", bufs=2))
        sbB = ctx.enter_context(tc.tile_pool(name="sbB", bufs=2))

        def gcols(k8, g):
            return slice(k8 * BS + g * NS, k8 * BS + (g + 1) * NS)

        def phase_a(g):
            pre_ps = psPre.tile([N, NS], FP32, tag="pre")
            for k8 in range(8):
                nc.tensor.matmul(pre_ps[:], wpt_sb[:, bass.ts(k8, N)],
                                 xt_sb[:, gcols(k8, g)],
                                 start=(k8 == 0), stop=(k8 == 7))
            th = sbA.tile([N, NS], FP32, tag="th")
            nc.scalar.activation(th[:], pre_ps[:], AF.Tanh)
            c = sbA.tile([N, NS], FP32, tag="c")
            nc.scalar.activation(c[:], th[:], AF.Sin, bias=pi2_sb[0:N, :],
                                 scale=PI_4)
            lc = sbA.tile([N, NS], FP32, tag="lc")
            nc.scalar.activation(lc[:], c[:], AF.Ln)
            ab = sbA.tile([N, NS], FP32, tag="ab")
            nc.scalar.activation(ab[:], th[:], AF.Abs, scale=PI_4)
            sa = sbA.tile([N, NS], FP32, tag="sa")
            nc.scalar.activation(sa[:], ab[:], AF.Sin)
            ls = sbA.tile([N, NS], FP32, tag="ls")
            nc.scalar.activation(ls[:], sa[:], AF.Ln, bias=eps_sb[0:N, :])
            nn = sbA.tile([N, NS], FP32, tag="nn")
            nc.scalar.activation(nn[:], th[:], AF.Sign)
            nh = sbA.tile([N, NS], FP16, tag="nh")
            nc.vector.tensor_scalar(nh[:], nn[:], -0.5, 0.5,
                                    mybir.AluOpType.mult, mybir.AluOpType.add)
            # dl128 quadrant layout (matches el row blocks): [0:10] hi of
            # ls-lc, [32:42] hi of lc, [64:74] lo of ls-lc, [96:106] lo of lc
            lsmc = sbA.tile([N, NS], FP32, tag="lsmc")
            nc.vector.tensor_sub(lsmc[:], ls[:], lc[:])
            dl = sbA.tile([128, NS], FP16, tag="dl")
            nc.vector.memset(dl[:], 0.0)
            nc.vector.tensor_copy(dl[0:N, :], lsmc[:])
            nc.vector.tensor_copy(dl[32:32 + N, :], lc[:])
            hi_a = sbA.tile([N, NS], FP32, tag="hi_a")
            nc.vector.tensor_copy(hi_a[:], dl[0:N, :])
            hi_b = sbA.tile([N, NS], FP32, tag="hi_b")
            nc.vector.tensor_copy(hi_b[:], dl[32:32 + N, :])
            nc.vector.tensor_sub(dl[64:64 + N, :], lsmc[:], hi_a[:])
            nc.vector.tensor_sub(dl[96:96 + N, :], lc[:], hi_b[:])
            return dl, nh

        def phase_lp_k8(g, dl, nh, k8):
                L_ps = psL.tile([128, NS], FP32, tag="L")
                nc.tensor.matmul(L_ps[:], el_sb[:, bass.ts(k8, 128)], dl[:],
                                 start=True, stop=True)
                P_ps = psP.tile([128, NS], FP32, tag="P")
                nc.tensor.matmul(P_ps[:], el_sb[0:N, bass.ts(k8, 128)], nh[:],
                                 start=True, stop=True)
                eL = sbB.tile([128, NS], FP16, tag="eL")
                nc.scalar.activation(eL[:], L_ps[:], AF.Exp)
                # Exact parity: fp16 round-to-even at 2048 keeps only even
                # integers, so d = (P+2048) - fp16(P+2048) = +-(P mod 2)
                # and sign = 1 - 2*d^2 = (-1)^P.
                p16 = sbB.tile([128, NS], FP16, tag="p16")
                nc.scalar.activation(p16[:], P_ps[:], AF.Copy, bias=2048.0)
                d = sbB.tile([128, NS], FP32, tag="d")
                nc.vector.scalar_tensor_tensor(
                    d[:], P_ps[:], 2048.0, p16[:],
                    mybir.AluOpType.add, mybir.AluOpType.subtract)
                d2 = sbB.tile([128, NS], FP32, tag="d2")
                nc.vector.tensor_mul(d2[:], d[:], d[:])
                sgn = sbB.tile([128, NS], FP16, tag="sgn")
                nc.vector.tensor_scalar(sgn[:], d2[:], -2.0, 1.0,
                                        mybir.AluOpType.mult,
                                        mybir.AluOpType.add)
                nc.vector.tensor_mul(psiT[:, gcols(k8, g)], sgn[:], eL[:])

        def phase_b(g, post_j8=None):
            o_ps = psO.tile([C, NS], FP32, tag="o")
            pts = []

            def issue_o(j8):
                nc.tensor.matmul(o_ps[:], g_sb[:, bass.ts(j8, C)],
                                 pts[j8][:], start=(j8 == 0), stop=(j8 == 7))

            for j8 in range(8):
                re_ps = psB.tile([128, NS], FP32, tag="re")
                for k8 in range(8):
                    nc.tensor.matmul(
                        re_ps[:],
                        utr_sb[:, k8 * AMP + j8 * 128:
                               k8 * AMP + (j8 + 1) * 128],
                        psiT[:, gcols(k8, g)],
                        start=(k8 == 0), stop=(k8 == 7))
                im_ps = psB.tile([128, NS], FP32, tag="im")
                for k8 in range(8):
                    nc.tensor.matmul(
                        im_ps[:],
                        uti_sb[:, k8 * AMP + j8 * 128:
                               k8 * AMP + (j8 + 1) * 128],
                        psiT[:, gcols(k8, g)],
                        start=(k8 == 0), stop=(k8 == 7))
                sq_re = sbB.tile([128, NS], FP16, tag="sqre")
                nc.scalar.activation(sq_re[:], re_ps[:], AF.Square)
                sq_im = sbB.tile([128, NS], FP16, tag="sqim")
                nc.scalar.activation(sq_im[:], im_ps[:], AF.Square)
                pt = sbB.tile([128, NS], FP16, tag="pt")
                nc.vector.tensor_add(pt[:], sq_re[:], sq_im[:])
                pts.append(pt)
                if j8 >= 1:
                    issue_o(j8 - 1)
                if post_j8 is not None:
                    post_j8(j8)
            issue_o(7)
            osb = sbB.tile([C, NS], FP32, tag="osb")
            nc.vector.tensor_scalar_add(osb[:], o_ps[:], b_sb[:, 0:1])
            nc.gpsimd.dma_start(out_d[:, bass.ts(g, NS)], osb[:])

        dln = [phase_a(g) for g in range(NG)]
        for k8 in range(8):
            phase_lp_k8(0, *dln[0], k8)
        phase_b(0, post_j8=lambda j8: phase_lp_k8(1, *dln[1], j8))
        phase_b(1)

    nc.compile()
    return nc


def _host_arrays(x, W_proj, weights, W_out, b_out):
    UT = _build_unitary(weights)
    utr = np.ascontiguousarray(UT.real.astype(np.float16))
    uti = np.ascontiguousarray(UT.imag.astype(np.float16))

    bits = (np.arange(AMP)[None, :] >> (N - 1 - np.arange(N)[:, None])) & 1
    zs = (1.0 - 2.0 * bits).astype(np.float32)            # (10, 1024)
    g = np.ascontiguousarray(
        (zs.T @ W_out.T.astype(np.float32)).astype(np.float16))
    el = np.zeros((128, AMP), np.float16)
    el[0:N] = bits
    el[32:32 + N] = 1.0
    el[64:64 + N] = bits
    el[96:96 + N] = 1.0
    el = np.ascontiguousarray(el)
    wpt = np.ascontiguousarray(W_proj.T)                  # (1024, 10)
    b = np.ascontiguousarray(b_out.reshape(C, 1))
    xt = np.ascontiguousarray(x.T)                        # (1024, 8192)
    return dict(xt=xt, utr=utr, uti=uti, wpt=wpt, g=g, b=b, el=el)


_PROG = None


def kernel(x, W_proj, weights, W_out, b_out):
    global _PROG, LAST_EXEC_NS
    x = np.asarray(x, np.float32)
    W_proj = np.asarray(W_proj, np.float32)
    W_out = np.asarray(W_out, np.float32)
    b_out = np.asarray(b_out, np.float32)

    h = _host_arrays(x, W_proj, weights, W_out, b_out)
    xt = h.pop("xt")

    if _PROG is None:
        _PROG = _build_program()

    in_maps = []
    for ci in range(NCORES):
        m = dict(h)
        m["xt"] = np.ascontiguousarray(xt[:, ci * BS:(ci + 1) * BS])
        in_maps.append(m)

    want_trace = bool(int(os.environ.get("KERNEL_TRACE", "0")))
    try:
        rr = run_bass_kernel_spmd(
            _PROG, in_maps, core_ids=list(range(NCORES)), trace=want_trace)
    except Exception:
        if not want_trace:
            raise
        rr = run_bass_kernel_spmd(
            _PROG, in_maps, core_ids=list(range(NCORES)), trace=False)
    LAST_EXEC_NS = rr.exec_time_ns

    outT = np.concatenate([np.asarray(r["out"]) for r in rr.results], axis=1)
    return np.ascontiguousarray(outT.T).astype(np.float32)
